# revision 33
# baseline (speedup 1.0000x reference)
"""Trainium2 Bass kernel for nn_MultiHeadAttention (b=2, n=4096, d=512, h=8, hd=64).

Sharding: 8 cores; core c handles batch b=c//4 and head pair j=c%4
(heads 2j, 2j+1). Tensor-parallel heads: each core computes a partial
output-projection y_part; host sums the 4 partials per batch and adds bo.

v3 design (all matmul paths bf16, fp32 PSUM accumulate; measured rel err
5.4e-3 vs the 2e-2 gate):
  - x arrives HOST-TRANSPOSED (xT [128, 4, n] bf16): no on-device x
    transposes and no PSUM->SBUF transpose drains.
  - Q/K projections land in qsb/ksb [128, n] with head0 dims on
    partitions 0-63 and head1 on 64-127, UNREPLICATED: the two heads'
    score matmuls (K=64 each) are emitted adjacently and row-pack onto
    the PE array concurrently (lhsT base partitions 0/64 auto-derive
    tile_position row groups (0,0)/(64,0)), so no replication DMAs.
  - V is computed directly in [keys, vdims] layout with x-stationary
    matmuls (out tokens x vdims), killing the separate V transposes.
  - scores supers alternate 3/2 chunks across two PSUM pools
    (psA 3 banks + psB 2 banks) giving a double-buffered PE->ACT
    pipeline; ps_acc (2 banks) holds the two attnv accumulators and
    ps_y (1 bank) the output-projection tiles. 8 banks exactly.
  - softmax sums ride as a ones-column in v_aug (row 64 of the attnv
    accumulator); reciprocal (bf16) is broadcast across partitions via
    a DRAM bounce whose latency hides under the DEFERRED finalize
    (output projection runs ~8 supers into the next q-block's sweep).
    NOTE: gpsimd partition_broadcast instead of the bounce produced
    NaN on hardware (it reads partition 0 regardless of the input AP's
    base partition, unlike the interpreter) - do not revisit.
  - Q projections for blocks 1-7 are deferred into the super stream
    (emitted one q-block ahead) to shorten the ACT-starved phase 1.
  - phase 1 (K/V projections) is interleaved with both heads' qb0
    sweeps as key blocks become ready, so ACT starts ~immediately.
  - NOT viable (tried): K-split row-group packing of attnv/projections
    (two concurrent matmuls may NOT accumulate into the same PSUM bank:
    runtime INTERNAL error); 1024-wide moving operands (matmul output
    must fit one 2KB PSUM bank); fp8 DoubleRow attnv (needs interleaved
    moving layout ACT cannot produce).
"""

import numpy as np

B, N, D, H, HD = 2, 4096, 512, 8, 64
NBLK = N // 512        # 8 token blocks
KC = N // 128          # 32 key chunks
QB = N // 512          # 8 q-blocks

# per-qb super schedule over 64 items [(h0,c),(h1,c) for c in 0..31]:
# sizes alternate psA(3)/psB(2): [3,2]*12 + [2,2] = 26 supers covering 64
SUPER_SIZES = [3, 2] * 12 + [2, 2]
assert sum(SUPER_SIZES) == 64 and len(SUPER_SIZES) % 2 == 0

_CACHE = {}
ABLATE = "base"  # timing-ablation knob, used only by scratch drivers


def _build_nc(loop_n=None):
    """Build the SPMD kernel. loop_n wraps the body in a hardware For loop
    (used only for timing amplification, never for the graded path)."""
    import contextlib

    import concourse.bass as bass
    import concourse.mybir as mybir
    import concourse.tile as tile
    from concourse import bacc

    F32 = mybir.dt.float32
    BF16 = mybir.dt.bfloat16
    EXP = mybir.ActivationFunctionType.Exp

    nc = bacc.Bacc("TRN2", target_bir_lowering=False, debug=False, num_devices=8)

    xT_d = nc.dram_tensor("xT2", [128, 4, N], BF16, kind="ExternalInput")
    w_d = {}
    for nm in ("wq2", "wk2", "wv2"):
        w_d[nm] = nc.dram_tensor(nm, [128, 4, 128], BF16, kind="ExternalInput")
    woT_d = nc.dram_tensor("woT", [128, 512], BF16, kind="ExternalInput")
    ones_d = nc.dram_tensor("ones", [128, 1], BF16, kind="ExternalInput")
    ident_d = nc.dram_tensor("ident", [128, 128], BF16, kind="ExternalInput")
    y_d = nc.dram_tensor("y_part", [N, D], BF16, kind="ExternalOutput")
    recip_dram = nc.dram_tensor("recip_scratch", [2, N], BF16, kind="Internal")

    with tile.TileContext(nc) as tc:
        with (
            tc.tile_pool(name="singles", bufs=1) as singles,
            tc.tile_pool(name="sb_exp", bufs=6) as sb_exp,
            tc.tile_pool(name="sb_scr", bufs=3) as sb_scr,
            tc.tile_pool(name="sb_vt", bufs=2) as sb_vt,
            tc.tile_pool(name="sb_y", bufs=3) as sb_y,
            tc.tile_pool(name="psA", bufs=1, space="PSUM") as psA,
            tc.tile_pool(name="psB", bufs=1, space="PSUM") as psB,
            tc.tile_pool(name="ps_acc", bufs=2, space="PSUM") as ps_acc,
            tc.tile_pool(name="ps_y", bufs=1, space="PSUM") as ps_y,
        ):
            loop_ctx = (
                tc.For_i(0, loop_n, 1) if loop_n else contextlib.nullcontext()
            )
            with loop_ctx:
                # ones first so the ACT warm can start immediately
                ones_sb = singles.tile([128, 1], BF16)
                nc.sync.dma_start(out=ones_sb, in_=ones_d.ap())
                warm = singles.tile([1, 1], F32)
                nc.scalar.activation(out=warm, in_=ones_sb[0:1, 0:1], func=EXP)
                wt = {}
                for nm in ("wq2", "wk2", "wv2"):
                    wt[nm] = singles.tile(
                        [128, 4, 128], BF16, tag=f"w_{nm}", name=f"wt_{nm}"
                    )
                    nc.sync.dma_start(out=wt[nm], in_=w_d[nm].ap())
                woT = singles.tile([128, 512], BF16)
                nc.sync.dma_start(out=woT, in_=woT_d.ap())
                ident = singles.tile([128, 128], BF16)
                nc.sync.dma_start(out=ident, in_=ident_d.ap())

                xT_sb = singles.tile([128, 4, N], BF16, tag="xT", name="xT_sb")
                for jj in range(NBLK):
                    js = slice(jj * 512, (jj + 1) * 512)
                    nc.sync.dma_start(
                        out=xT_sb[:, :, js], in_=xT_d.ap()[:, :, js]
                    )

                qsb = singles.tile([128, N], BF16, tag="qsb", name="qsb")
                ksb = singles.tile([128, N], BF16, tag="ksb", name="ksb")
                v_aug = [singles.tile([128, KC, 65], BF16, tag=f"vaug{h}",
                                      name=f"vaug{h}") for h in range(2)]
                ot2 = singles.tile([128, N], BF16)
                recip_b = singles.tile([128, N], BF16)

                # ones column of v_aug: DVE broadcasts along the free dim
                # (step-0 read AP)
                for h in range(2):
                    ones_rd = bass.AP(
                        tensor=ones_sb.tensor, offset=ones_sb.offset,
                        ap=[ones_sb.ap[0], [0, KC], [1, 1]],
                    )
                    nc.vector.tensor_copy(out=v_aug[h][:, :, HD:65], in_=ones_rd)

                # -------- helpers --------
                def finalize(qb, spare_psum=False):
                    qs = slice(qb * 512, (qb + 1) * 512)
                    nc.vector.tensor_mul(ot2[:, qs], ot2[:, qs], recip_b[:, qs])
                    for i, nt in enumerate(range(4 * qb, 4 * qb + 4)):
                        if spare_psum and i < 2:
                            # the super pools are drained by now; borrow their
                            # banks so the last finalize's matmuls overlap
                            pool, cap = (psA, 3) if i == 0 else (psB, 2)
                            psy = pool.tile(
                                [128, cap, 512], F32, tag="ps_s", name="psy_sp"
                            )[:, 0, :]
                        else:
                            psy = ps_y.tile([128, 512], F32, tag="psy", name="psy")
                        nc.tensor.matmul(
                            psy, ot2[:, nt * 128:(nt + 1) * 128], woT,
                            start=True, stop=True,
                        )
                        yb = sb_y.tile([128, 512], BF16, tag="yb", name="yb")
                        nc.vector.tensor_copy(out=yb, in_=psy)
                        nc.sync.dma_start(
                            out=y_d.ap()[nt * 128:(nt + 1) * 128, :], in_=yb
                        )

                accs = {}

                def tails(qb):
                    qs = slice(qb * 512, (qb + 1) * 512)
                    for h in range(2):
                        acc = accs.pop(h)
                        scr = sb_scr.tile([128, 512], BF16, tag="scr", name="scr")
                        if h == 0:
                            nc.vector.tensor_copy(
                                out=ot2[0:64, qs], in_=acc[0:64, :]
                            )
                        else:
                            nc.vector.tensor_copy(
                                out=scr[0:64, :], in_=acc[0:64, :]
                            )
                            nc.sync.dma_start(
                                out=ot2[64:128, qs], in_=scr[0:64, :]
                            )
                        # reciprocal of softmax sums (bf16), broadcast across
                        # 64 partitions via a DRAM bounce (latency hidden by
                        # the deferred finalize)
                        rr = scr[64:65, :]
                        with nc.allow_low_precision(reason="softmax recip bf16"):
                            nc.vector.reciprocal(out=rr, in_=acc[64:65, :])
                        nc.sync.dma_start(
                            out=recip_dram.ap()[h:h + 1, qs], in_=rr
                        )
                        rb = bass.AP(
                            tensor=recip_dram, offset=h * N + qb * 512,
                            ap=[[0, 64], [1, 512]],
                        )
                        nc.sync.dma_start(
                            out=recip_b[h * 64:(h + 1) * 64, qs], in_=rb
                        )

                def items_of(off, w):
                    return [((off + i) % 2, (off + i) // 2) for i in range(w)]

                def emit_super(qb, off, w, use_a):
                    qs = slice(qb * 512, (qb + 1) * 512)
                    pool, cap = (psA, 3) if use_a else (psB, 2)
                    S = pool.tile([128, cap, 512], F32, tag="ps_s", name="ps_s")
                    for i, (h, c) in enumerate(items_of(off, w)):
                        hs = slice(h * 64, (h + 1) * 64)
                        nc.tensor.matmul(
                            S[:, i, :],
                            ksb[hs, c * 128:(c + 1) * 128],
                            qsb[hs, qs],
                            start=True, stop=True,
                        )
                    E = sb_exp.tile([128, 3, 512], BF16, tag="expT", name="expT")
                    sl = slice(0, 1) if ABLATE == "tiny_exp" else slice(0, 512)
                    nc.scalar.activation(
                        out=E[:, 0:w, sl], in_=S[:, 0:w, sl], func=EXP, scale=0.125,
                    )
                    return E

                LOOKAHEAD = 3
                DEFER_FLUSHES = 8
                pending = []
                fin_sched = {}   # flush_count -> qb to finalize
                state = {"flushes": 0}

                def flush_one():
                    qb, off, w, E, last = pending.pop(0)
                    for i, (h, c) in enumerate(items_of(off, w)):
                        if h not in accs:
                            accs[h] = ps_acc.tile(
                                [128, 512], F32, tag="acc", name=f"ps_o{h}"
                            )
                        nc.tensor.matmul(
                            accs[h][0:65, :], v_aug[h][:, c, :], E[:, i, :],
                            start=(c == 0), stop=(c == KC - 1),
                        )
                    state["flushes"] += 1
                    if last:
                        tails(qb)
                        fin_sched[state["flushes"] + DEFER_FLUSHES] = qb
                    fq = fin_sched.pop(state["flushes"], None)
                    if fq is not None:
                        finalize(fq)

                def proj_mms(out_ap, wtile, js):
                    for dc in range(4):
                        nc.tensor.matmul(
                            out_ap, wtile[:, dc, :], xT_sb[:, dc, js],
                            start=(dc == 0), stop=(dc == 3),
                        )

                def proj_q(qb):
                    # deferred Q projection for q-block qb (blocks 1..7)
                    js = slice(qb * 512, (qb + 1) * 512)
                    Pq = psB.tile([128, 2, 512], F32, tag="ps_s", name="proj_q")
                    proj_mms(Pq[:, 0, :], wt["wq2"], js)
                    nc.vector.tensor_copy(out=qsb[:, js], in_=Pq[:, 0, :])

                def push_super(qb, si, off, w):
                    E = emit_super(qb, off, w, use_a=(si % 2 == 0))
                    if len(pending) >= LOOKAHEAD:
                        flush_one()
                    pending.append(
                        (qb, off, w, E, si == len(SUPER_SIZES) - 1)
                    )
                    if si == 13 and qb < QB - 1:
                        proj_q(qb + 1)

                # qb0 supers become ready when the token block holding their
                # last chunk has been projected
                ready = {}
                off = 0
                for si, w in enumerate(SUPER_SIZES):
                    last_c = (off + w - 1) // 2
                    ready.setdefault(last_c // 4, []).append((si, off, w))
                    off += w

                # ---- phase 1: project k/v per token block (q only for
                # block 0; the rest are deferred into the super stream),
                # interleave both heads' qb0 sweeps ----
                for jj in range(NBLK):
                    js = slice(jj * 512, (jj + 1) * 512)
                    # alternate proj tiles between psA (even blocks, 3 slots)
                    # and psB (odd blocks, 2 slots) so consecutive blocks
                    # don't serialize on a single-buffered PSUM tile.
                    # V: W-stationary projection [vdims, tokens], DVE drain to
                    # bf16, then PE-transpose into [tokens, vdims] reusing a
                    # drained slot (bf16 bitcast view).
                    if jj % 2 == 0:
                        P = psA.tile([128, 3, 512], F32, tag="ps_s", name="proj")
                        k_sl, v_sl, t_sl = P[:, 1, :], P[:, 2, :], P[:, 0, :]
                        if jj == 0:
                            proj_mms(t_sl, wt["wq2"], js)
                    else:
                        P = psB.tile([128, 2, 512], F32, tag="ps_s", name="projB")
                        k_sl, v_sl, t_sl = P[:, 0, :], P[:, 1, :], P[:, 0, :]
                    proj_mms(k_sl, wt["wk2"], js)
                    proj_mms(v_sl, wt["wv2"], js)
                    if jj == 0:
                        nc.vector.tensor_copy(out=qsb[:, js], in_=t_sl)
                    nc.vector.tensor_copy(out=ksb[:, js], in_=k_sl)
                    vt = sb_vt.tile([128, 512], BF16, tag="vt", name="vt")
                    nc.vector.tensor_copy(out=vt, in_=v_sl)
                    pvb = t_sl.bitcast(BF16)         # [128, 1024] bf16 view
                    for tt in range(4):
                        nc.tensor.transpose(
                            pvb[:, tt * 128:(tt + 1) * 128],
                            vt[:, tt * 128:(tt + 1) * 128], ident,
                        )
                    for h in range(2):
                        src = bass.AP(
                            tensor=pvb.tensor, offset=pvb.offset + h * 64,
                            ap=[pvb.ap[0], [128, 4], [1, 64]],
                        )
                        nc.vector.tensor_copy(
                            out=v_aug[h][:, 4 * jj:4 * jj + 4, 0:HD], in_=src
                        )
                    for si, s_off, w in ready.get(jj, []):
                        push_super(0, si, s_off, w)

                # ---- phase 2: remaining q-blocks, software-pipelined ----
                for qb in range(1, QB):
                    off = 0
                    for si, w in enumerate(SUPER_SIZES):
                        push_super(qb, si, off, w)
                        off += w
                while pending:
                    flush_one()
                for k in sorted(fin_sched):
                    finalize(fin_sched[k], spare_psum=(k == max(fin_sched)))

    nc.compile()
    return nc


def _prep_in_maps(x, Wq, Wk, Wv, Wo):
    from ml_dtypes import bfloat16

    x = np.asarray(x, dtype=np.float32)
    Wq = np.asarray(Wq, dtype=np.float32)
    Wk = np.asarray(Wk, dtype=np.float32)
    Wv = np.asarray(Wv, dtype=np.float32)
    Wo = np.asarray(Wo, dtype=np.float32)
    ones = np.ones((128, 1), bfloat16)
    in_maps = []
    xT_b = [
        np.ascontiguousarray(
            x[b].T.reshape(4, 128, N).transpose(1, 0, 2)
        ).astype(bfloat16)
        for b in range(B)
    ]
    for c in range(8):
        b, j = c // 4, c % 4
        rows = slice(128 * j, 128 * (j + 1))
        m = {
            "xT2": xT_b[b],
            "ones": ones,
            "ident": np.eye(128, dtype=bfloat16),
            "woT": np.ascontiguousarray(Wo[:, rows].T).astype(bfloat16),
        }
        for nm, W in (("wq2", Wq), ("wk2", Wk), ("wv2", Wv)):
            A = W[rows]                     # [128, 512] rows = h1(64) | h2(64)
            # lhsT layout [p=d-within-chunk, c=d-chunk, k=out-col], contiguous
            m[nm] = np.ascontiguousarray(
                A.reshape(128, 4, 128).transpose(2, 1, 0)
            ).astype(bfloat16)
        in_maps.append(m)
    return in_maps


def _make_runner(nc):
    """Persistent jitted SPMD executor (mirrors bass2jax.run_bass_via_pjrt)
    so repeat kernel() calls skip re-tracing/re-jitting."""
    import jax
    from jax.sharding import Mesh, PartitionSpec
    from jax.experimental.shard_map import shard_map
    import concourse.mybir as mybir
    from concourse import bass2jax

    bass2jax.install_neuronx_cc_hook()
    n_cores = 8
    partition_name = nc.partition_id_tensor.name if nc.partition_id_tensor else None
    in_names, out_names, out_avals, zero_shapes = [], [], [], []
    for alloc in nc.m.functions[0].allocations:
        if not isinstance(alloc, mybir.MemoryLocationSet):
            continue
        name = alloc.memorylocations[0].name
        if alloc.kind == "ExternalInput":
            if name != partition_name:
                in_names.append(name)
        elif alloc.kind == "ExternalOutput":
            shape = tuple(alloc.tensor_shape)
            dtype = mybir.dt.np(alloc.dtype)
            out_names.append(name)
            out_avals.append(jax.core.ShapedArray(shape, dtype))
            zero_shapes.append((shape, dtype))
    n_params = len(in_names)
    all_names = in_names + out_names
    if partition_name is not None:
        all_names = all_names + [partition_name]

    def _body(*args):
        operands = list(args)
        if partition_name is not None:
            operands.append(bass2jax.partition_id_tensor())
        return tuple(bass2jax._bass_exec_p.bind(
            *operands,
            out_avals=tuple(out_avals),
            in_names=tuple(all_names),
            out_names=tuple(out_names),
            lowering_input_output_aliases=(),
            sim_require_finite=True,
            sim_require_nnan=True,
            nc=nc,
        ))

    mesh = Mesh(np.asarray(jax.devices()[:n_cores]), ("core",))
    n_outs = len(out_names)
    sharded = jax.jit(
        shard_map(
            _body, mesh=mesh,
            in_specs=(PartitionSpec("core"),) * (n_params + n_outs),
            out_specs=(PartitionSpec("core"),) * n_outs,
            check_rep=False,
        ),
        keep_unused=True,
    )
    zeros = [np.zeros((n_cores * s[0], *s[1:]), d) for s, d in zero_shapes]

    def run(in_maps):
        concat_in = [
            np.concatenate([np.asarray(in_maps[c][nm]) for c in range(n_cores)],
                           axis=0)
            for nm in in_names
        ]
        outs = sharded(*concat_in, *zeros)
        arr = np.asarray(outs[out_names.index("y_part")])
        return arr.reshape(n_cores, N, D)

    return run


def kernel(x, Wq, Wk, Wv, Wo, bo):
    if "nc" not in _CACHE:
        _CACHE["nc"] = _build_nc()
    nc = _CACHE["nc"]
    in_maps = _prep_in_maps(x, Wq, Wk, Wv, Wo)
    try:
        if "runner" not in _CACHE:
            _CACHE["runner"] = _make_runner(nc)
        parts = _CACHE["runner"](in_maps)
    except Exception:
        from concourse.bass_utils import run_bass_kernel_spmd
        res = run_bass_kernel_spmd(nc, in_maps, core_ids=list(range(8)))
        parts = np.stack([res.results[c]["y_part"] for c in range(8)])
    parts = np.asarray(parts, dtype=np.float32)
    y = np.zeros((B, N, D), np.float32)
    for c in range(8):
        y[c // 4] += parts[c]
    y += np.asarray(bo, dtype=np.float32)[None, None, :]
    return y


# revision 34
# speedup vs baseline: 1.0663x; 1.0663x over previous
"""Trainium2 Bass kernel for nn_MultiHeadAttention (b=2, n=4096, d=512, h=8, hd=64).

Sharding: 8 cores; core c handles batch b=c//4 and head pair j=c%4
(heads 2j, 2j+1). Tensor-parallel heads: each core computes a partial
output-projection y_part; host sums the 4 partials per batch and adds bo.

v3 design (all matmul paths bf16, fp32 PSUM accumulate; measured rel err
5.4e-3 vs the 2e-2 gate):
  - x arrives HOST-TRANSPOSED (xT [128, 4, n] bf16): no on-device x
    transposes and no PSUM->SBUF transpose drains.
  - Q/K projections land in qsb/ksb [128, n] with head0 dims on
    partitions 0-63 and head1 on 64-127, UNREPLICATED: the two heads'
    score matmuls (K=64 each) are emitted adjacently and row-pack onto
    the PE array concurrently (lhsT base partitions 0/64 auto-derive
    tile_position row groups (0,0)/(64,0)), so no replication DMAs.
  - V is computed directly in [keys, vdims] layout with x-stationary
    matmuls (out tokens x vdims), killing the separate V transposes.
  - scores supers alternate 3/2 chunks across two PSUM pools
    (psA 3 banks + psB 2 banks) giving a double-buffered PE->ACT
    pipeline; ps_acc (2 banks) holds the two attnv accumulators and
    ps_y (1 bank) the output-projection tiles. 8 banks exactly.
  - softmax sums ride as a ones-column in v_aug (row 64 of the attnv
    accumulator); reciprocal (bf16) is broadcast across partitions via
    a DRAM bounce whose latency hides under the DEFERRED finalize
    (output projection runs ~8 supers into the next q-block's sweep).
    NOTE: gpsimd partition_broadcast instead of the bounce produced
    NaN on hardware (it reads partition 0 regardless of the input AP's
    base partition, unlike the interpreter) - do not revisit.
  - Q projections for blocks 1-7 are deferred into the super stream
    (emitted one q-block ahead) to shorten the ACT-starved phase 1.
  - phase 1 (K/V projections) is interleaved with both heads' qb0
    sweeps as key blocks become ready, so ACT starts ~immediately.
  - NOT viable (tried): K-split row-group packing of attnv/projections
    (two concurrent matmuls may NOT accumulate into the same PSUM bank:
    runtime INTERNAL error); 1024-wide moving operands (matmul output
    must fit one 2KB PSUM bank); fp8 DoubleRow attnv (needs interleaved
    moving layout ACT cannot produce).
"""

import numpy as np

B, N, D, H, HD = 2, 4096, 512, 8, 64
NBLK = N // 512        # 8 token blocks
KC = N // 128          # 32 key chunks
QB = N // 512          # 8 q-blocks

# per-qb super schedule over 64 items [(h0,c),(h1,c) for c in 0..31]:
# sizes alternate psA(3)/psB(2): [3,2]*12 + [2,2] = 26 supers covering 64
SUPER_SIZES = [3, 2] * 12 + [2, 2]
assert sum(SUPER_SIZES) == 64 and len(SUPER_SIZES) % 2 == 0

_CACHE = {}
ABLATE = "base"  # timing-ablation knob, used only by scratch drivers


def _build_nc(loop_n=None):
    """Build the SPMD kernel. loop_n wraps the body in a hardware For loop
    (used only for timing amplification, never for the graded path)."""
    import contextlib

    import concourse.bass as bass
    import concourse.mybir as mybir
    import concourse.tile as tile
    from concourse import bacc

    F32 = mybir.dt.float32
    BF16 = mybir.dt.bfloat16
    EXP = mybir.ActivationFunctionType.Exp

    nc = bacc.Bacc("TRN2", target_bir_lowering=False, debug=False, num_devices=8)

    xT_d = nc.dram_tensor("xT2", [128, 4, N], BF16, kind="ExternalInput")
    w_d = {}
    for nm in ("wq2", "wk2", "wv2"):
        w_d[nm] = nc.dram_tensor(nm, [128, 4, 128], BF16, kind="ExternalInput")
    woT_d = nc.dram_tensor("woT", [128, 512], BF16, kind="ExternalInput")
    ones_d = nc.dram_tensor("ones", [128, 1], BF16, kind="ExternalInput")
    ident_d = nc.dram_tensor("ident", [128, 128], BF16, kind="ExternalInput")
    y_d = nc.dram_tensor("y_part", [N, D], BF16, kind="ExternalOutput")
    recip_dram = nc.dram_tensor("recip_scratch", [2, N], BF16, kind="Internal")

    with tile.TileContext(nc) as tc:
        with (
            tc.tile_pool(name="singles", bufs=1) as singles,
            tc.tile_pool(name="sb_exp", bufs=6) as sb_exp,
            tc.tile_pool(name="sb_scr", bufs=3) as sb_scr,
            tc.tile_pool(name="sb_vt", bufs=2) as sb_vt,
            tc.tile_pool(name="sb_y", bufs=3) as sb_y,
            tc.tile_pool(name="psA", bufs=1, space="PSUM") as psA,
            tc.tile_pool(name="psB", bufs=1, space="PSUM") as psB,
            tc.tile_pool(name="ps_acc", bufs=2, space="PSUM") as ps_acc,
            tc.tile_pool(name="ps_y", bufs=1, space="PSUM") as ps_y,
        ):
            loop_ctx = (
                tc.For_i(0, loop_n, 1) if loop_n else contextlib.nullcontext()
            )
            with loop_ctx:
                # ones first so the ACT warm can start immediately
                ones_sb = singles.tile([128, 1], BF16)
                nc.sync.dma_start(out=ones_sb, in_=ones_d.ap())
                warm = singles.tile([1, 1], F32)
                nc.scalar.activation(out=warm, in_=ones_sb[0:1, 0:1], func=EXP)
                wt = {}
                for nm in ("wq2", "wk2", "wv2"):
                    wt[nm] = singles.tile(
                        [128, 4, 128], BF16, tag=f"w_{nm}", name=f"wt_{nm}"
                    )
                    nc.sync.dma_start(out=wt[nm], in_=w_d[nm].ap())
                woT = singles.tile([128, 512], BF16)
                nc.sync.dma_start(out=woT, in_=woT_d.ap())
                ident = singles.tile([128, 128], BF16)
                nc.sync.dma_start(out=ident, in_=ident_d.ap())

                xT_sb = singles.tile([128, 4, N], BF16, tag="xT", name="xT_sb")
                for jj in range(NBLK):
                    js = slice(jj * 512, (jj + 1) * 512)
                    nc.sync.dma_start(
                        out=xT_sb[:, :, js], in_=xT_d.ap()[:, :, js]
                    )

                qsb = singles.tile([128, N], BF16, tag="qsb", name="qsb")
                ksb = singles.tile([128, N], BF16, tag="ksb", name="ksb")
                v_aug = [singles.tile([128, KC, 65], BF16, tag=f"vaug{h}",
                                      name=f"vaug{h}") for h in range(2)]
                ot2 = singles.tile([128, N], BF16)
                recip_b = singles.tile([128, N], BF16)

                # ones column of v_aug: DVE broadcasts along the free dim
                # (step-0 read AP)
                for h in range(2):
                    ones_rd = bass.AP(
                        tensor=ones_sb.tensor, offset=ones_sb.offset,
                        ap=[ones_sb.ap[0], [0, KC], [1, 1]],
                    )
                    nc.vector.tensor_copy(out=v_aug[h][:, :, HD:65], in_=ones_rd)

                # -------- helpers --------
                def finalize(qb, spare_psum=False):
                    qs = slice(qb * 512, (qb + 1) * 512)
                    nc.vector.tensor_mul(ot2[:, qs], ot2[:, qs], recip_b[:, qs])
                    for i, nt in enumerate(range(4 * qb, 4 * qb + 4)):
                        if spare_psum and i < 2:
                            # the super pools are drained by now; borrow their
                            # banks so the last finalize's matmuls overlap
                            pool, cap = (psA, 3) if i == 0 else (psB, 2)
                            psy = pool.tile(
                                [128, cap, 512], F32, tag="ps_s", name="psy_sp"
                            )[:, 0, :]
                        else:
                            psy = ps_y.tile([128, 512], F32, tag="psy", name="psy")
                        nc.tensor.matmul(
                            psy, ot2[:, nt * 128:(nt + 1) * 128], woT,
                            start=True, stop=True,
                        )
                        yb = sb_y.tile([128, 512], BF16, tag="yb", name="yb")
                        nc.vector.tensor_copy(out=yb, in_=psy)
                        nc.sync.dma_start(
                            out=y_d.ap()[nt * 128:(nt + 1) * 128, :], in_=yb
                        )

                accs = {}

                def tails(qb):
                    qs = slice(qb * 512, (qb + 1) * 512)
                    for h in range(2):
                        acc = accs.pop(h)
                        scr = sb_scr.tile([128, 512], BF16, tag="scr", name="scr")
                        if h == 0:
                            nc.vector.tensor_copy(
                                out=ot2[0:64, qs], in_=acc[0:64, :]
                            )
                        else:
                            nc.vector.tensor_copy(
                                out=scr[0:64, :], in_=acc[0:64, :]
                            )
                            nc.sync.dma_start(
                                out=ot2[64:128, qs], in_=scr[0:64, :]
                            )
                        # reciprocal of softmax sums (bf16), broadcast across
                        # 64 partitions via a DRAM bounce (latency hidden by
                        # the deferred finalize)
                        rr = scr[64:65, :]
                        with nc.allow_low_precision(reason="softmax recip bf16"):
                            nc.vector.reciprocal(out=rr, in_=acc[64:65, :])
                        nc.sync.dma_start(
                            out=recip_dram.ap()[h:h + 1, qs], in_=rr
                        )
                        rb = bass.AP(
                            tensor=recip_dram, offset=h * N + qb * 512,
                            ap=[[0, 64], [1, 512]],
                        )
                        nc.sync.dma_start(
                            out=recip_b[h * 64:(h + 1) * 64, qs], in_=rb
                        )

                def items_of(off, w):
                    return [((off + i) % 2, (off + i) // 2) for i in range(w)]

                def emit_super(qb, off, w, use_a):
                    qs = slice(qb * 512, (qb + 1) * 512)
                    pool, cap = (psA, 3) if use_a else (psB, 2)
                    S = pool.tile([128, cap, 512], F32, tag="ps_s", name="ps_s")
                    for i, (h, c) in enumerate(items_of(off, w)):
                        hs = slice(h * 64, (h + 1) * 64)
                        nc.tensor.matmul(
                            S[:, i, :],
                            ksb[hs, c * 128:(c + 1) * 128],
                            qsb[hs, qs],
                            start=True, stop=True,
                        )
                    E = sb_exp.tile([128, 3, 512], BF16, tag="expT", name="expT")
                    sl = slice(0, 1) if ABLATE == "tiny_exp" else slice(0, 512)
                    nc.scalar.activation(
                        out=E[:, 0:w, sl], in_=S[:, 0:w, sl], func=EXP, scale=0.125,
                    )
                    return E

                LOOKAHEAD = 3
                DEFER_FLUSHES = 8
                pending = []
                fin_sched = {}   # flush_count -> qb to finalize
                state = {"flushes": 0}

                def flush_one():
                    qb, off, w, E, last = pending.pop(0)
                    for i, (h, c) in enumerate(items_of(off, w)):
                        if h not in accs:
                            accs[h] = ps_acc.tile(
                                [128, 512], F32, tag="acc", name=f"ps_o{h}"
                            )
                        nc.tensor.matmul(
                            accs[h][0:65, :], v_aug[h][:, c, :], E[:, i, :],
                            start=(c == 0), stop=(c == KC - 1),
                        )
                    state["flushes"] += 1
                    if last:
                        tails(qb)
                        fin_sched[state["flushes"] + DEFER_FLUSHES] = qb
                    fq = fin_sched.pop(state["flushes"], None)
                    if fq is not None:
                        finalize(fq)

                def proj_mms(out_ap, wtile, js):
                    for dc in range(4):
                        nc.tensor.matmul(
                            out_ap, wtile[:, dc, :], xT_sb[:, dc, js],
                            start=(dc == 0), stop=(dc == 3),
                        )

                def proj_q(qb):
                    # deferred Q projection for q-block qb (blocks 1..7)
                    js = slice(qb * 512, (qb + 1) * 512)
                    Pq = psB.tile([128, 2, 512], F32, tag="ps_s", name="proj_q")
                    proj_mms(Pq[:, 0, :], wt["wq2"], js)
                    nc.vector.tensor_copy(out=qsb[:, js], in_=Pq[:, 0, :])

                def push_super(qb, si, off, w):
                    E = emit_super(qb, off, w, use_a=(si % 2 == 0))
                    if len(pending) >= LOOKAHEAD:
                        flush_one()
                    pending.append(
                        (qb, off, w, E, si == len(SUPER_SIZES) - 1)
                    )
                    if si == 13 and qb < QB - 1:
                        proj_q(qb + 1)

                # qb0 supers become ready when the token block holding their
                # last chunk has been projected
                ready = {}
                off = 0
                for si, w in enumerate(SUPER_SIZES):
                    last_c = (off + w - 1) // 2
                    ready.setdefault(last_c // 4, []).append((si, off, w))
                    off += w

                # ---- phase 1: project k/v per token block (q only for
                # block 0; the rest are deferred into the super stream),
                # interleave both heads' qb0 sweeps ----
                for jj in range(NBLK):
                    js = slice(jj * 512, (jj + 1) * 512)
                    P = psA.tile([128, 3, 512], F32, tag="ps_s", name="proj")
                    if jj == 0:
                        proj_mms(P[:, 0, :], wt["wq2"], js)
                    proj_mms(P[:, 1, :], wt["wk2"], js)
                    # V: W-stationary projection [vdims, tokens], DVE drain to
                    # bf16, then PE-transpose into [tokens, vdims] using the
                    # spare q slot of P (fewer PE instructions than the
                    # x-stationary direct form: 16 vs 32 per block)
                    proj_mms(P[:, 2, :], wt["wv2"], js)
                    if jj == 0:
                        nc.vector.tensor_copy(out=qsb[:, js], in_=P[:, 0, :])
                    nc.vector.tensor_copy(out=ksb[:, js], in_=P[:, 1, :])
                    vt = sb_vt.tile([128, 512], BF16, tag="vt", name="vt")
                    nc.vector.tensor_copy(out=vt, in_=P[:, 2, :])
                    pvb = P[:, 0, :].bitcast(BF16)   # [128, 1024] bf16 view
                    for tt in range(4):
                        nc.tensor.transpose(
                            pvb[:, tt * 128:(tt + 1) * 128],
                            vt[:, tt * 128:(tt + 1) * 128], ident,
                        )
                    for h in range(2):
                        src = bass.AP(
                            tensor=pvb.tensor, offset=pvb.offset + h * 64,
                            ap=[pvb.ap[0], [128, 4], [1, 64]],
                        )
                        nc.vector.tensor_copy(
                            out=v_aug[h][:, 4 * jj:4 * jj + 4, 0:HD], in_=src
                        )
                    for si, s_off, w in ready.get(jj, []):
                        push_super(0, si, s_off, w)

                # ---- phase 2: remaining q-blocks, software-pipelined ----
                for qb in range(1, QB):
                    off = 0
                    for si, w in enumerate(SUPER_SIZES):
                        push_super(qb, si, off, w)
                        off += w
                while pending:
                    flush_one()
                for k in sorted(fin_sched):
                    finalize(fin_sched[k], spare_psum=(k == max(fin_sched)))

    nc.compile()
    return nc


def _prep_in_maps(x, Wq, Wk, Wv, Wo):
    from ml_dtypes import bfloat16

    x = np.asarray(x, dtype=np.float32)
    Wq = np.asarray(Wq, dtype=np.float32)
    Wk = np.asarray(Wk, dtype=np.float32)
    Wv = np.asarray(Wv, dtype=np.float32)
    Wo = np.asarray(Wo, dtype=np.float32)
    ones = np.ones((128, 1), bfloat16)
    in_maps = []
    xT_b = [
        np.ascontiguousarray(
            x[b].T.reshape(4, 128, N).transpose(1, 0, 2)
        ).astype(bfloat16)
        for b in range(B)
    ]
    for c in range(8):
        b, j = c // 4, c % 4
        rows = slice(128 * j, 128 * (j + 1))
        m = {
            "xT2": xT_b[b],
            "ones": ones,
            "ident": np.eye(128, dtype=bfloat16),
            "woT": np.ascontiguousarray(Wo[:, rows].T).astype(bfloat16),
        }
        for nm, W in (("wq2", Wq), ("wk2", Wk), ("wv2", Wv)):
            A = W[rows]                     # [128, 512] rows = h1(64) | h2(64)
            # lhsT layout [p=d-within-chunk, c=d-chunk, k=out-col], contiguous
            m[nm] = np.ascontiguousarray(
                A.reshape(128, 4, 128).transpose(2, 1, 0)
            ).astype(bfloat16)
        in_maps.append(m)
    return in_maps


def _make_runner(nc):
    """Persistent jitted SPMD executor (mirrors bass2jax.run_bass_via_pjrt)
    so repeat kernel() calls skip re-tracing/re-jitting."""
    import jax
    from jax.sharding import Mesh, PartitionSpec
    from jax.experimental.shard_map import shard_map
    import concourse.mybir as mybir
    from concourse import bass2jax

    bass2jax.install_neuronx_cc_hook()
    n_cores = 8
    partition_name = nc.partition_id_tensor.name if nc.partition_id_tensor else None
    in_names, out_names, out_avals, zero_shapes = [], [], [], []
    for alloc in nc.m.functions[0].allocations:
        if not isinstance(alloc, mybir.MemoryLocationSet):
            continue
        name = alloc.memorylocations[0].name
        if alloc.kind == "ExternalInput":
            if name != partition_name:
                in_names.append(name)
        elif alloc.kind == "ExternalOutput":
            shape = tuple(alloc.tensor_shape)
            dtype = mybir.dt.np(alloc.dtype)
            out_names.append(name)
            out_avals.append(jax.core.ShapedArray(shape, dtype))
            zero_shapes.append((shape, dtype))
    n_params = len(in_names)
    all_names = in_names + out_names
    if partition_name is not None:
        all_names = all_names + [partition_name]

    def _body(*args):
        operands = list(args)
        if partition_name is not None:
            operands.append(bass2jax.partition_id_tensor())
        return tuple(bass2jax._bass_exec_p.bind(
            *operands,
            out_avals=tuple(out_avals),
            in_names=tuple(all_names),
            out_names=tuple(out_names),
            lowering_input_output_aliases=(),
            sim_require_finite=True,
            sim_require_nnan=True,
            nc=nc,
        ))

    mesh = Mesh(np.asarray(jax.devices()[:n_cores]), ("core",))
    n_outs = len(out_names)
    sharded = jax.jit(
        shard_map(
            _body, mesh=mesh,
            in_specs=(PartitionSpec("core"),) * (n_params + n_outs),
            out_specs=(PartitionSpec("core"),) * n_outs,
            check_rep=False,
        ),
        keep_unused=True,
    )
    zeros = [np.zeros((n_cores * s[0], *s[1:]), d) for s, d in zero_shapes]

    def run(in_maps):
        concat_in = [
            np.concatenate([np.asarray(in_maps[c][nm]) for c in range(n_cores)],
                           axis=0)
            for nm in in_names
        ]
        outs = sharded(*concat_in, *zeros)
        arr = np.asarray(outs[out_names.index("y_part")])
        return arr.reshape(n_cores, N, D)

    return run


def kernel(x, Wq, Wk, Wv, Wo, bo):
    if "nc" not in _CACHE:
        _CACHE["nc"] = _build_nc()
    nc = _CACHE["nc"]
    in_maps = _prep_in_maps(x, Wq, Wk, Wv, Wo)
    try:
        if "runner" not in _CACHE:
            _CACHE["runner"] = _make_runner(nc)
        parts = _CACHE["runner"](in_maps)
    except Exception:
        from concourse.bass_utils import run_bass_kernel_spmd
        res = run_bass_kernel_spmd(nc, in_maps, core_ids=list(range(8)))
        parts = np.stack([res.results[c]["y_part"] for c in range(8)])
    parts = np.asarray(parts, dtype=np.float32)
    y = np.zeros((B, N, D), np.float32)
    for c in range(8):
        y[c // 4] += parts[c]
    y += np.asarray(bo, dtype=np.float32)[None, None, :]
    return y
